# revision 23
# baseline (speedup 1.0000x reference)
"""Trainium2 Bass kernel for nn_MixedFeedForward (shared MLP + 16 per-ns-token MLPs).

Sharding (8 NeuronCores, SPMD, no collectives):
  - shared path: data-parallel over batch -> core i runs the shared MLP over
    x[i, :1024, :].
  - ns path: expert-parallel -> core i runs experts {2i, 2i+1}, each over the
    8 batches' single ns token for that expert.
Each core writes a disjoint slice of the output; the host assembles.

Per-core kernel (all matmuls bf16; f32 inputs cast on-chip):
  L1: psum[128F, 512tok] = W1_blk(lhsT) x xT_blk; fused bias+Gelu on ScalarE
      -> bf16 hT[F, tok] resident in SBUF.
  L2 shared (transposed out): psum[128D, 512tok] = W2_blk(lhsT) x hT_blk;
      fused bias via ScalarE Identity -> f32 outT[D, tok]; host transposes.
  L2 experts: psum[8tok, 512D] = heT(lhsT) x W2e_blk; bias on VectorE.
Weights stream via 1MB f32 staging pieces -> DVE/ACT cast to bf16 -> PE.
Expert rounds are emitted one f-block ahead of the shared path so their DMA
prefetches under shared compute; expert L2 is interleaved into shared L2.
"""

import os
import sys
import numpy as np

P = 128
D_MODEL, D_FF = 1024, 4096
SEQ_TOK, NS_TOK, BATCH = 1024, 16, 8
SEQ_LEN = SEQ_TOK + NS_TOK
N_CORES = 8
E_PER_CORE = 2
KO1 = D_MODEL // P      # 8  k-chunks when contracting over d_model
KO2 = D_FF // P         # 32 k-chunks when contracting over d_ff
FBLK = D_FF // 512      # 8  f-blocks (512 wide)
TBLK = SEQ_TOK // 512   # 2  token blocks (512 wide)

_state = {}


def _ensure_axon_profile_hook():
    """Some agent images lack antenv.axon_hooks; provide a shim so
    run_bass_kernel_spmd(trace=True) can capture NTFF profiles via the
    libaxon_pjrt C ABI (same mechanism as trn_agent_boot)."""
    try:
        import antenv.axon_hooks  # noqa: F401
        return
    except ImportError:
        pass
    import contextlib
    import ctypes
    import types

    so_path = "/opt/axon/libaxon_pjrt.so"
    hook = None
    if os.path.exists(so_path):
        try:
            lib = ctypes.CDLL(so_path)
            if hasattr(lib, "axon_start_nrt_profile"):
                lib.axon_start_nrt_profile.argtypes = [
                    ctypes.POINTER(ctypes.c_int64), ctypes.c_size_t]
                lib.axon_start_nrt_profile.restype = ctypes.c_int64
                lib.axon_stop_nrt_profile.argtypes = [ctypes.c_char_p]
                lib.axon_stop_nrt_profile.restype = ctypes.c_int64

                @contextlib.contextmanager
                def _hook(output_dir, device_ids):
                    import jax
                    jax.devices()
                    if device_ids:
                        ids = (ctypes.c_int64 * len(device_ids))(*device_ids)
                        rc = lib.axon_start_nrt_profile(ids, len(device_ids))
                    else:
                        rc = lib.axon_start_nrt_profile(None, 0)
                    if rc != 0:
                        raise RuntimeError(f"axon_start_nrt_profile rc={rc}")
                    try:
                        yield
                    finally:
                        n = lib.axon_stop_nrt_profile(str(output_dir).encode())
                        print(f"profile: {n} file(s) written to {output_dir}",
                              file=sys.stderr)

                hook = _hook
        except OSError:
            pass

    mod = types.ModuleType("antenv.axon_hooks")
    _store = {"hook": hook}
    mod.set_axon_ntff_profile_hook = lambda h: _store.__setitem__("hook", h)
    mod.get_axon_ntff_profile_hook = lambda: _store["hook"]
    sys.modules["antenv.axon_hooks"] = mod


_ensure_axon_profile_hook()


def _build():
    import concourse.mybir as mybir
    import concourse.tile as tile
    from concourse import bacc

    f32 = mybir.dt.float32
    bf16 = mybir.dt.bfloat16
    AF = mybir.ActivationFunctionType

    nc = bacc.Bacc(None, target_bir_lowering=False, debug=False)

    # piece-major DRAM layouts: every staging DMA below is one fully
    # contiguous 1 MiB read (8 KiB per partition)
    xTp = nc.dram_tensor("xTp", [TBLK, 2, P, KO1 // 2, 512], f32, kind="ExternalInput")
    xnsT = nc.dram_tensor("xnsT", [P, KO1, E_PER_CORE * BATCH], f32, kind="ExternalInput")
    w1sp = nc.dram_tensor("w1sp", [FBLK, 2, P, KO1 // 2, 512], f32, kind="ExternalInput")
    w2sp = nc.dram_tensor("w2sp", [4, 4, P, KO2 // 4, 256], f32, kind="ExternalInput")
    b1s = nc.dram_tensor("b1s", [P, KO2], f32, kind="ExternalInput")
    b2s = nc.dram_tensor("b2s", [P, KO1], f32, kind="ExternalInput")
    w1ep = nc.dram_tensor("w1ep", [E_PER_CORE, FBLK, 2, P, KO1 // 2, 512], f32,
                          kind="ExternalInput")
    w2ep = nc.dram_tensor("w2ep", [E_PER_CORE, 2, 4, 2, P, KO1 // 2, 512], f32,
                          kind="ExternalInput")
    b1e = nc.dram_tensor("b1e", [E_PER_CORE, P, KO2], f32, kind="ExternalInput")
    b2e = nc.dram_tensor("b2e", [E_PER_CORE, BATCH, D_MODEL], f32, kind="ExternalInput")
    outsT = nc.dram_tensor("outsT", [D_MODEL, SEQ_TOK], f32, kind="ExternalOutput")
    outns = nc.dram_tensor("outns", [E_PER_CORE * BATCH, D_MODEL], f32, kind="ExternalOutput")

    cast_idx = [0]

    with tile.TileContext(nc) as tc:
        with tc.tile_pool(name="main", bufs=1) as pool, \
             tc.tile_pool(name="psum", bufs=1, space="PSUM") as pp:

            def cast(dst, src):
                # all f32->bf16 casts on DVE (2x single-src mode); keeps ACT's
                # FIFO free for gelu so hT is never queued behind weight casts
                cast_idx[0] += 1
                nc.vector.tensor_copy(out=dst, in_=src)

            # ---- constants ------------------------------------------------
            b1s_sb = pool.tile([P, KO2], f32, tag="b1s", bufs=1)
            nc.sync.dma_start(out=b1s_sb, in_=b1s[:])
            b2s_sb = pool.tile([P, KO1], f32, tag="b2s", bufs=1)
            nc.sync.dma_start(out=b2s_sb, in_=b2s[:])
            b1e_sb, b2e_sb = [], []
            for le in range(E_PER_CORE):
                t1 = pool.tile([P, KO2], f32, tag=f"b1e{le}", bufs=1, name=f"b1e_sb{le}")
                nc.sync.dma_start(out=t1, in_=b1e[le])
                b1e_sb.append(t1)
                t2 = pool.tile([BATCH, D_MODEL], f32, tag=f"b2e{le}", bufs=1, name=f"b2e_sb{le}")
                nc.sync.dma_start(out=t2, in_=b2e[le])
                b2e_sb.append(t2)

            # ---- ns tokens (tiny; first so expert L1 can start early) -----
            xnsf = pool.tile([P, KO1, E_PER_CORE * BATCH], f32, tag="xnsf", bufs=1)
            nc.sync.dma_start(out=xnsf, in_=xnsT[:])
            xnsb = pool.tile([P, KO1, E_PER_CORE * BATCH], bf16, tag="xnsb", bufs=1)
            nc.vector.tensor_copy(out=xnsb, in_=xnsf)

            # ---- persistent activations ----------------------------------
            xb = pool.tile([P, KO1, SEQ_TOK], bf16, tag="xb", bufs=1)
            hT = pool.tile([P, KO2, SEQ_TOK], bf16, tag="hT", bufs=1)
            heT = []
            for le in range(E_PER_CORE):
                t = pool.tile([P, KO2, BATCH], bf16, tag=f"heT{le}", bufs=1,
                              name=f"heT{le}")
                heT.append(t)

            # staging helpers: 1 MiB f32 pieces, shared slot pools
            WSTAGE_BUFS = 5

            def stage_w1_like(piece_aps, key):
                """load [P, KO1, 512] f32 as 2 contiguous 1MiB pieces, cast to
                one bf16 tile."""
                wb = pool.tile([P, KO1, 512], bf16, tag="wb", bufs=4,
                               name=f"wb_{key}")
                for pc, piece in enumerate(piece_aps):
                    wf = pool.tile([P, KO1 // 2, 512], f32, tag="wstage",
                                   bufs=WSTAGE_BUFS, name=f"wf_{key}_{pc}")
                    nc.sync.dma_start(out=wf, in_=piece)
                    cast(wb[:, pc * (KO1 // 2):(pc + 1) * (KO1 // 2), :], wf)
                return wb

            def expert_l1_round(le, fb):
                web = stage_w1_like([w1ep[le, fb, 0], w1ep[le, fb, 1]],
                                    f"e{le}_{fb}")
                for fs in range(4):
                    fc = fb * 4 + fs
                    pse = pp.tile([P, BATCH], f32, tag="pse1", bufs=2,
                                  name=f"pse1_{le}_{fc}")
                    for k in range(KO1):
                        nc.tensor.matmul(
                            pse,
                            web[:, k, fs * 128:(fs + 1) * 128],
                            xnsb[:, k, le * BATCH:(le + 1) * BATCH],
                            start=(k == 0), stop=(k == KO1 - 1))
                    nc.scalar.activation(
                        heT[le][:, fc, :], pse, AF.Gelu,
                        bias=b1e_sb[le][:, fc:fc + 1])

            # ---- warm-up: expert L1 f-block 0 while x streams in ----------
            expert_l1_round(0, 0)
            w1b_next = stage_w1_like([w1sp[0, 0], w1sp[0, 1]], "s0")
            # x load + cast, token-block major so L1 tb=0 can start early
            for tb in range(TBLK):
                for kh in range(2):
                    xf = pool.tile([P, KO1 // 2, 512], f32, tag="wstage",
                                   bufs=WSTAGE_BUFS, name=f"xf{tb}_{kh}")
                    nc.sync.dma_start(out=xf, in_=xTp[tb, kh])
                    nc.vector.tensor_copy(
                        out=xb[:, kh * (KO1 // 2):(kh + 1) * (KO1 // 2),
                               tb * 512:(tb + 1) * 512],
                        in_=xf)
            expert_l1_round(1, 0)

            # ---- layer 1 main loop ---------------------------------------
            for fb in range(FBLK):
                w1b = w1b_next
                for tb in range(TBLK):
                    for fs in range(4):
                        fc = fb * 4 + fs
                        ps1 = pp.tile([P, 512], f32, tag="ps1", bufs=2,
                                      name=f"ps1_{fc}_{tb}")
                        for k in range(KO1):
                            nc.tensor.matmul(
                                ps1,
                                w1b[:, k, fs * 128:(fs + 1) * 128],
                                xb[:, k, tb * 512:(tb + 1) * 512],
                                start=(k == 0), stop=(k == KO1 - 1))
                        nc.scalar.activation(
                            hT[:, fc, tb * 512:(tb + 1) * 512], ps1, AF.Gelu,
                            bias=b1s_sb[:, fc:fc + 1])
                # prefetch one f-block ahead: experts first (PE hits them next)
                if fb + 1 < FBLK:
                    for le in range(E_PER_CORE):
                        expert_l1_round(le, fb + 1)
                    w1b_next = stage_w1_like(
                        [w1sp[fb + 1, 0], w1sp[fb + 1, 1]], f"s{fb + 1}")

            # ---- layer 2 -------------------------------------------------
            # shared path, transposed output: 256-wide d slices, double-buffered
            # bf16 W2 cache; expert L2 interleaved between slices.
            def fill_w2c(ds):
                w2c = pool.tile([P, KO2, 256], bf16, tag=f"w2c{ds % 2}", bufs=1,
                                name=f"w2c_{ds}")
                for pc in range(4):
                    w2f = pool.tile([P, KO2 // 4, 256], f32, tag="wstage",
                                    bufs=WSTAGE_BUFS, name=f"w2f_{ds}_{pc}")
                    nc.sync.dma_start(out=w2f, in_=w2sp[ds, pc])
                    cast(w2c[:, pc * (KO2 // 4):(pc + 1) * (KO2 // 4), :], w2f)
                return w2c

            def shared_l2_slice(ds, w2c):
                for dc in range(2):
                    dchunk = ds * 2 + dc
                    for tb in range(TBLK):
                        ps2 = pp.tile([P, 512], f32, tag="ps2", bufs=2,
                                      name=f"ps2_{dchunk}_{tb}")
                        for k in range(KO2):
                            nc.tensor.matmul(
                                ps2,
                                w2c[:, k, dc * 128:(dc + 1) * 128],
                                hT[:, k, tb * 512:(tb + 1) * 512],
                                start=(k == 0), stop=(k == KO2 - 1))
                        ot = pool.tile([P, 512], f32, tag="ot", bufs=2,
                                       name=f"ot_{dchunk}_{tb}")
                        nc.scalar.activation(ot, ps2, AF.Identity,
                                             bias=b2s_sb[:, dchunk:dchunk + 1])
                        nc.sync.dma_start(
                            out=outsT[dchunk * 128:(dchunk + 1) * 128,
                                      tb * 512:(tb + 1) * 512],
                            in_=ot)

            def expert_l2_half(le, db):
                dsl = slice(db * 512, (db + 1) * 512)
                webs = []
                for pc in range(4):
                    web2 = pool.tile([P, KO1, 512], bf16, tag="wb", bufs=4,
                                     name=f"w2eb_{le}_{db}_{pc}")
                    for hp in range(2):
                        w2ef = pool.tile([P, KO1 // 2, 512], f32, tag="wstage",
                                         bufs=WSTAGE_BUFS,
                                         name=f"w2ef_{le}_{db}_{pc}_{hp}")
                        nc.sync.dma_start(out=w2ef, in_=w2ep[le, db, pc, hp])
                        cast(web2[:, hp * (KO1 // 2):(hp + 1) * (KO1 // 2), :], w2ef)
                    webs.append(web2)
                pse2 = pp.tile([BATCH, 512], f32, tag="pse2", bufs=2,
                               name=f"pse2_{le}_{db}")
                for k in range(KO2):
                    nc.tensor.matmul(
                        pse2,
                        heT[le][:, k, :],
                        webs[k // KO1][:, k % KO1, :],
                        start=(k == 0), stop=(k == KO2 - 1))
                obe = pool.tile([BATCH, 512], f32, tag="obe", bufs=2,
                                name=f"obe_{le}_{db}")
                nc.vector.tensor_add(out=obe, in0=pse2, in1=b2e_sb[le][:, dsl])
                nc.sync.dma_start(out=outns[le * BATCH:(le + 1) * BATCH, dsl],
                                  in_=obe)

            # PE order: ds0, e0a, ds1, e0b, ds2, e1a, ds3, e1b — the last DMA
            # bytes (e1b weights) feed matmuls that consume them at arrival
            # rate, and ds3 (long prefetched) hides the e1b stream under PE.
            w2c0 = fill_w2c(0)
            w2c1 = fill_w2c(1)
            shared_l2_slice(0, w2c0)
            expert_l2_half(0, 0)
            w2c2 = fill_w2c(2)
            shared_l2_slice(1, w2c1)
            expert_l2_half(0, 1)
            w2c3 = fill_w2c(3)
            shared_l2_slice(2, w2c2)
            expert_l2_half(1, 0)
            shared_l2_slice(3, w2c3)
            expert_l2_half(1, 1)

    nc.compile()
    return nc


def _get_nc():
    if "nc" not in _state:
        _state["nc"] = _build()
    return _state["nc"]


def _np(a):
    return np.ascontiguousarray(np.asarray(a, dtype=np.float32))


def kernel(x, W1_seq, b1_seq, W2_seq, b2_seq, W1_ns, b1_ns, W2_ns, b2_ns,
           seq_token_count):
    from concourse.bass_utils import run_bass_kernel_spmd

    assert int(seq_token_count) == SEQ_TOK
    x = _np(x)
    W1_seq, b1_seq = _np(W1_seq), _np(b1_seq)
    W2_seq, b2_seq = _np(W2_seq), _np(b2_seq)
    W1_ns, b1_ns = _np(W1_ns), _np(b1_ns)
    W2_ns, b2_ns = _np(W2_ns), _np(b2_ns)

    nc = _get_nc()

    # host-side (lossless) re-layouts: contraction dim on partitions, then
    # piece-major packing so each device DMA is one contiguous 1MiB read
    w1s_h = W1_seq.reshape(KO1, P, D_FF).transpose(1, 0, 2)         # [P, KO1, F]
    w1sp_h = np.ascontiguousarray(
        w1s_h.reshape(P, 2, KO1 // 2, FBLK, 512).transpose(3, 1, 0, 2, 4))
    w2s_h = W2_seq.reshape(KO2, P, D_MODEL).transpose(1, 0, 2)      # [P, KO2, D]
    w2sp_h = np.ascontiguousarray(
        w2s_h.reshape(P, 4, KO2 // 4, 4, 256).transpose(3, 1, 0, 2, 4))
    b1s_h = np.ascontiguousarray(b1_seq.reshape(KO2, P).T)          # [P, KO2]
    b2s_h = np.ascontiguousarray(b2_seq.reshape(KO1, P).T)          # [P, KO1]

    in_maps = []
    for i in range(N_CORES):
        xT_h = x[i, :SEQ_TOK, :].T.reshape(KO1, P, SEQ_TOK).transpose(1, 0, 2)
        xTp_h = np.ascontiguousarray(
            xT_h.reshape(P, 2, KO1 // 2, TBLK, 512).transpose(3, 1, 0, 2, 4))
        xns = x[:, SEQ_TOK + 2 * i:SEQ_TOK + 2 * i + 2, :]          # [B, 2, D]
        xnsT_h = np.ascontiguousarray(
            xns.transpose(2, 1, 0).reshape(KO1, P, E_PER_CORE, BATCH)
            .transpose(1, 0, 2, 3).reshape(P, KO1, E_PER_CORE * BATCH))
        w1e_h = W1_ns[2 * i:2 * i + 2].reshape(E_PER_CORE, KO1, P, D_FF) \
            .transpose(0, 2, 1, 3)                                  # [E, P, KO1, F]
        w1ep_h = np.ascontiguousarray(
            w1e_h.reshape(E_PER_CORE, P, 2, KO1 // 2, FBLK, 512)
            .transpose(0, 4, 2, 1, 3, 5))
        w2e_h = W2_ns[2 * i:2 * i + 2].reshape(E_PER_CORE, KO2, P, D_MODEL) \
            .transpose(0, 2, 1, 3)                                  # [E, P, KO2, D]
        w2ep_h = np.ascontiguousarray(
            w2e_h.reshape(E_PER_CORE, P, 4, 2, KO1 // 2, 2, 512)
            .transpose(0, 5, 2, 3, 1, 4, 6))
        b1e_h = np.ascontiguousarray(
            b1_ns[2 * i:2 * i + 2].reshape(E_PER_CORE, KO2, P).transpose(0, 2, 1))
        b2e_h = np.ascontiguousarray(
            np.broadcast_to(b2_ns[2 * i:2 * i + 2, None, :],
                            (E_PER_CORE, BATCH, D_MODEL)))
        in_maps.append({
            "xTp": xTp_h, "xnsT": xnsT_h,
            "w1sp": w1sp_h, "w2sp": w2sp_h, "b1s": b1s_h, "b2s": b2s_h,
            "w1ep": w1ep_h, "w2ep": w2ep_h, "b1e": b1e_h, "b2e": b2e_h,
        })

    trace = bool(int(os.environ.get("KERNEL_TRACE", "0")))
    kw = {}
    if trace:
        kw["trace"] = True
        tc_env = os.environ.get("KERNEL_TRACE_CORES", "0")
        kw["trace_cores"] = [int(c) for c in tc_env.split(",")]
    res = run_bass_kernel_spmd(nc, in_maps, list(range(N_CORES)), **kw)
    _state["last_result"] = res

    out = np.empty((BATCH, SEQ_LEN, D_MODEL), np.float32)
    for i in range(N_CORES):
        out[i, :SEQ_TOK, :] = res.results[i]["outsT"].T
        ns = res.results[i]["outns"].reshape(E_PER_CORE, BATCH, D_MODEL)
        out[:, SEQ_TOK + 2 * i, :] = ns[0]
        out[:, SEQ_TOK + 2 * i + 1, :] = ns[1]
    return out
